# revision 31
# baseline (speedup 1.0000x reference)
"""GAT 2-layer kernel for 8 Trainium2 NeuronCores (Bass/Tile).

Sharding: nodes (feat and output rows) across 8 cores (12544-aligned shards);
edges partitioned by destination-node owner so segment softmax / scatter-add
are local; source-node features provided by an AllGather of the per-core
feature-table shard (the "halo" exchange).

Per core:
  Phase A: own1[i] = [h1(64)|el1(2)|er1(2)] packed bf16 256B rows for the
    core's own 12544 nodes; h1 = feat_shard @ W1 on the PE (bf16), el/er
    folded into extra matmul columns (W1 @ al^T etc).
  AllGather -> table1 (100352 rows, node order).
  Edge phase (layer 1): edges grouped by (src-bank, dst) into degree-sorted
    tiles of 128 (dst, bank) pairs; per group: dma_gather of source rows from
    the <=25088-row bank slice (int16 index limit), scores
    x = exp(leaky_relu(el_src + er_dst + maskbias)), weighted message sums
    reduced per tile, partial rows [out(64)|s(2)] dma_scatter_add'ed into a
    local accumulator keyed by local dst id. Self-loop edges excluded here.
  Normalize 1: o1 = relu((acc + selfcnt*x_self*h_self) / (s + selfcnt*x_self)
    + b1); fused layer-2 table build h2|el2|er2 = o1 @ W2ext -> own2.
  AllGather -> table2; same edge phase for layer 2 (f32 rows, 1 head);
  Normalize 2 -> output shard.
Both layers share the same graph layout, so all index/mask streams and the
tile schedule are shared.
"""

import numpy as np
import ml_dtypes

import concourse.bacc as bacc
import concourse.bass as bass
import concourse.mybir as mybir
import concourse.tile as tile
import concourse.tile_sem_assignment as _tsa
from concourse.bass_utils import run_bass_kernel_spmd
from concourse import library_config, bass_isa as _bass_isa
from concourse.masks import make_identity
from concourse.tile_scheduler import DMAInst as _DMAInst

# The Tile scheduler round-robins SWDGE DMA instructions over the 8 DMASW sem
# lanes independently of the instruction's SWDGE queue, but each lane's sem is
# locked to one queue by the runtime. Make the lane choice queue-aware
# (queue q -> lanes {q, q+4}) so multi-queue gathers/scatters are legal.
_orig_assign_tick = _tsa.TileClockTick._assign_tick


def _queue_aware_assign_tick(self, inst):
    if (isinstance(inst, _DMAInst)
            and inst.engine == mybir.EngineType.Pool
            and not isinstance(inst, _bass_isa.UserSyncedRemoteDMADescs)
            and self.swdge_sem_count == 8):
        q = int(getattr(inst, "queue_num", 0) or 0)
        rr = getattr(self, "_q_rr", None)
        if rr is None:
            rr = self._q_rr = {}
        sub = rr.get(q, 0)
        rr[q] = sub ^ 1
        self.next_sw_dma_idx = (q % 4) + 4 * sub
    return _orig_assign_tick(self, inst)


_tsa.TileClockTick._assign_tick = _queue_aware_assign_tick

f32 = mybir.dt.float32
bf16 = mybir.dt.bfloat16
i16 = mybir.dt.int16

LAST_RESULTS = None
NEG_SLOPE = 0.2
P = 128
NEG_BIG = -1e30

# problem dims
N_NODES = 100000
IN_DIM = 128
HID = 32
HEADS = 2
N_CORES = 8

OWN = 12544                  # per-core node rows (98 * 128)
GLOB = OWN * N_CORES         # 100352
BANKSZ = GLOB // 4           # 25088 (< int16 limit)
HCHUNK = OWN // 2            # 6272: AllGather chunk rows (2 chunks)
OWN_TILES = OWN // P         # 98
ACC_ROWS = OWN + P           # + garbage region
GARBAGE = OWN

MAXC = 28                    # max 128-idx chunks per gather call
HD = HEADS * HID             # 64
ROW1 = 128                   # bf16 cols: h(64) el(2) er(2) pad
ROW2 = 64                    # f32 cols: h2(32) el2(1) er2(1) pad
EL1 = HD                     # 64
EL2 = HID                    # 32


def _ceil(a, b):
    return (a + b - 1) // b


def _wrap_idx(arr):
    """[n] -> [128, n//16] int16 wrapped layout (replicated across Q7 cores)."""
    a = np.asarray(arr, np.int64).reshape(-1, 16).T.astype(np.int16)
    return np.tile(a, (8, 1))


MAXT_SG = 27                 # max tiles per supergroup (er/scatter call)

# pseudo-bank processing order: (dst-half, src-bank). Banks 0/1 (= table
# chunk 0) first within each half-phase so edge work can start after the
# first AllGather chunk; half 0 fully done by position 5 so normalize of
# rows [0, HCHUNK) overlaps the remaining supergroups.
PBS = [(0, 0), (0, 1), (1, 0), (1, 1), (0, 2), (0, 3), (1, 2), (1, 3)]
N_PB = len(PBS)


class Plan:
    def __init__(self, K_sched, groups):
        self.K_sched = K_sched          # [(pb, K)] per tile
        self.groups = groups            # [(pb, K, T)] uniform K per group
        self.total_gcols = sum(K * T for (_b, K, T) in groups)
        self.n_tiles = sum(T for (_b, K, T) in groups)
        self.gidx_cols = self.total_gcols * 8
        self.eidx_cols = self.n_tiles * 8
        # supergroups: [(pb, Tsg, [(K, T, tile_off_in_sg), ...])] —
        # consecutive same-pb groups batched for one er-gather + one scatter
        self.supergroups = []
        cur_b, cur_T, cur = None, 0, []
        for (b, K, T) in groups:
            if cur_b != b or cur_T + T > MAXT_SG:
                if cur:
                    self.supergroups.append((cur_b, cur_T, cur))
                cur_b, cur_T, cur = b, 0, []
            cur.append((K, T, cur_T))
            cur_T += T
        if cur:
            self.supergroups.append((cur_b, cur_T, cur))
        # per-supergroup stream offsets so emission can be split anywhere
        self.sg_offsets = []
        gcol = ecol = mcol = 0
        for (_pb, Tsg, sg_groups) in self.supergroups:
            self.sg_offsets.append((gcol, ecol, mcol))
            sgC = sum(K * T for (K, T, _t) in sg_groups)
            gcol += sgC * 8
            ecol += Tsg * 8
            mcol += sgC


def _build_plan(deg_sorted):
    """deg_sorted[c][pb]: descending per-pair degree arrays. Uniform
    schedule across cores."""
    n_cores = len(deg_sorted)
    K_sched = []
    for pb in range(N_PB):
        maxlen = max(len(deg_sorted[c][pb]) for c in range(n_cores))
        for i in range(_ceil(maxlen, P)):
            K = 0
            for c in range(n_cores):
                d = deg_sorted[c][pb]
                if i * P < len(d):
                    K = max(K, int(d[i * P]))
            if K > 0:
                K_sched.append((pb, K))
    groups = []
    i = 0
    while i < len(K_sched):
        b, K = K_sched[i]
        T = 1
        while (i + T < len(K_sched) and K_sched[i + T] == (b, K)
               and (T + 1) * K <= MAXC):
            T += 1
        groups.append((b, K, T))
        i += T
    return Plan(K_sched, groups)


def _core_streams(plan, pairs_by_bank):
    """Build gidx/eidx/sidx/mask streams for one core following the plan.

    eidx/sidx are half-local (rebased by the pseudo-bank's dst-half) so the
    er-gather / scatter-add touch only the matching half tensor."""
    gidx, eidx, sidx, mask = [], [], [], []
    bank_tile_counter = {b: 0 for b in range(N_PB)}
    for (b, K, T) in plan.groups:
        half = PBS[b][0]
        hbase = half * HCHUNK
        dstl_b, off_b, cnt_b, flat_b = pairs_by_bank[b]
        npairs = len(dstl_b)
        for _t in range(T):
            bi = bank_tile_counter[b]
            bank_tile_counter[b] += 1
            lo = bi * P
            members = np.arange(lo, lo + P)
            live = members < npairs
            mem_c = np.minimum(members, max(npairs - 1, 0))
            dl = np.where(live, dstl_b[mem_c], hbase)
            offs = off_b[mem_c]
            cnts = np.where(live, cnt_b[mem_c], 0)
            kk = np.arange(K)
            pos = offs[:, None] + kk[None, :]
            valid = kk[None, :] < cnts[:, None]
            slots = np.where(valid,
                             flat_b[np.minimum(pos, max(len(flat_b) - 1, 0))],
                             0)
            mk = np.where(valid, 0.0, NEG_BIG).astype(np.float32)
            gidx.append(slots.T.reshape(-1))          # chunk-major [K*P]
            mask.append(mk.T)                         # [K, P]
            eidx.append(np.where(live, dl - hbase, 0))
            sidx.append(np.where(live, dl - hbase, HCHUNK))
    gidx = np.concatenate(gidx)
    eidx = np.concatenate(eidx)
    sidx = np.concatenate(sidx)
    mask = np.concatenate(mask, axis=0).T.copy()      # [P, total_C]
    return (_wrap_idx(gidx), _wrap_idx(eidx), _wrap_idx(sidx),
            np.ascontiguousarray(mask, np.float32))


def _prep_host(src, dst):
    src = np.asarray(src, np.int64)
    dst = np.asarray(dst, np.int64)
    own = dst // OWN
    is_self = src == dst

    deg_sorted = [[None] * N_PB for _ in range(N_CORES)]
    per_core_pairs = []
    selfcnts = []
    for c in range(N_CORES):
        m = own == c
        sc = src[m]
        dc = dst[m] - c * OWN
        selfm = is_self[m]
        selfcnt = np.bincount(dc[selfm], minlength=OWN).astype(np.float32)
        selfcnt[selfcnt == 0] = 1.0
        selfcnts.append(selfcnt.reshape(OWN_TILES, P).T.copy())
        sc_ns = sc[~selfm]
        dc_ns = dc[~selfm]
        # chunked-AllGather table layout: node (c, j) sits at row
        # (j // HCHUNK) * 8*HCHUNK + c * HCHUNK + j % HCHUNK
        c_src = sc_ns // OWN
        j_src = sc_ns - c_src * OWN
        ci_src = j_src // HCHUNK
        row = ci_src * (N_CORES * HCHUNK) + c_src * HCHUNK + (j_src - ci_src * HCHUNK)
        bb = row // BANKSZ
        ll = row - bb * BANKSZ
        hh = dc_ns // HCHUNK          # dst half
        pairs_by_bank = {}
        for pb in range(N_PB):
            h, b = PBS[pb]
            mb = (bb == b) & (hh == h)
            dlb = dc_ns[mb]
            llb = ll[mb]
            order = np.argsort(dlb, kind="stable")
            dlb, llb = dlb[order], llb[order]
            uniq, inv, cnt = np.unique(dlb, return_inverse=True,
                                       return_counts=True)
            o2 = np.argsort(-cnt, kind="stable")
            uniq, cnt = uniq[o2], cnt[o2]
            # flatten member srcs in sorted-pair order (vectorized)
            rank = np.empty(len(o2), np.int64)
            rank[o2] = np.arange(len(o2))
            flat = llb[np.argsort(rank[inv], kind="stable")] \
                if len(uniq) else np.zeros(0, np.int64)
            off = np.zeros(len(uniq), np.int64)
            if len(uniq):
                off[1:] = np.cumsum(cnt)[:-1]
            pairs_by_bank[pb] = (uniq, off, cnt, flat)
            deg_sorted[c][pb] = cnt
        per_core_pairs.append(pairs_by_bank)

    plan = _build_plan(deg_sorted)
    streams = [_core_streams(plan, per_core_pairs[c]) for c in range(N_CORES)]
    return plan, streams, selfcnts


# ---------------------------------------------------------------------------

def _edge_phase(nc, pool, plan, *, gpool, table_full, table_own, accs,
                gidx_d, eidx_d, sidx_d, mask_d, row_w, row_dt, el_col, heads,
                tagp, qstate, sg_lo=0, sg_hi=None):
    D = HID
    HV = heads * D + heads
    if sg_hi is None:
        sg_hi = len(plan.supergroups)
    for si in range(sg_lo, sg_hi):
        (pb, Tsg, sg_groups) = plan.supergroups[si]
        gcol, ecol, mcol = plan.sg_offsets[si]
        half, b = PBS[pb]
        own_h = table_own[half * HCHUNK:(half + 1) * HCHUNK, :]
        acc = accs[half]
        eit = pool.tile([P, Tsg * 8], i16, tag=tagp + "eit")
        nc.sync.dma_start(eit[:], eidx_d[:, ecol:ecol + Tsg * 8])
        sit = pool.tile([P, Tsg * 8], i16, tag=tagp + "sit")
        nc.sync.dma_start(sit[:], sidx_d[:, ecol:ecol + Tsg * 8])
        er_g = gpool.tile([P, Tsg, row_w], row_dt, tag="sherg")
        nc.gpsimd.dma_gather(
            er_g[:], own_h, eit[:], Tsg * P, Tsg * P, row_w,
            single_packet=False, queue_num=2)
        er = pool.tile([P, Tsg, heads], f32, tag=tagp + "er")
        nc.scalar.copy(er[:],
                       er_g[:, :, el_col + heads:el_col + 2 * heads])
        staging = pool.tile([P, Tsg, HV], f32, tag=tagp + "st")
        sgC = sum(K * T for (K, T, _toff) in sg_groups)
        git_slab = pool.tile([P, sgC * 8], i16, tag=tagp + "git")
        nc.sync.dma_start(git_slab[:], gidx_d[:, gcol:gcol + sgC * 8])
        mkt_slab = pool.tile([P, sgC], f32, tag=tagp + "mkt")
        nc.sync.dma_start(mkt_slab[:], mask_d[:, mcol:mcol + sgC])
        sgoff = 0
        for (K, T, toff) in sg_groups:
            C = T * K
            git = git_slab[:, sgoff * 8:(sgoff + C) * 8]
            mkt = mkt_slab[:, sgoff:sgoff + C]
            g = gpool.tile([P, C, row_w], row_dt, tag="shg")
            nc.gpsimd.dma_gather(
                g[:], table_full[b * BANKSZ:(b + 1) * BANKSZ, :], git,
                C * P, C * P, row_w, single_packet=False,
                queue_num=qstate[0] % 2)
            qstate[0] += 1
            t = pool.tile([P, T, K, heads], f32, tag=tagp + "t")
            el_src = g[:, :, el_col:el_col + heads].rearrange(
                "p (t k) h -> p t k h", t=T)
            nc.vector.tensor_tensor(
                out=t[:], in0=el_src,
                in1=er[:, toff:toff + T].unsqueeze(2)
                    .broadcast_to([P, T, K, heads]),
                op=mybir.AluOpType.add)
            nc.vector.tensor_tensor(
                out=t[:], in0=t[:],
                in1=mkt.rearrange("p (t k) -> p t k", t=T).unsqueeze(3)
                    .broadcast_to([P, T, K, heads]),
                op=mybir.AluOpType.add)
            # exp(leaky(t)) == max(exp(t), exp(0.2*t)) -- keeps both
            # activations on the same Exp table of the idle scalar engine.
            x = pool.tile([P, T, K, heads], f32, tag=tagp + "x")
            nc.scalar.activation(x[:], t[:], mybir.ActivationFunctionType.Exp)
            x2 = pool.tile([P, T, K, heads], f32, tag=tagp + "x2")
            nc.scalar.activation(x2[:], t[:], mybir.ActivationFunctionType.Exp,
                                 scale=NEG_SLOPE)
            nc.vector.tensor_tensor(out=x[:], in0=x[:], in1=x2[:],
                                    op=mybir.AluOpType.max)
            m = pool.tile([P, C, heads * D], bf16, tag=tagp + "m")
            for h in range(heads):
                nc.vector.tensor_tensor(
                    out=m[:, :, h * D:(h + 1) * D],
                    in0=g[:, :, h * D:(h + 1) * D].rearrange(
                        "p (t k) d -> p t k d", t=T),
                    in1=x[:, :, :, h].unsqueeze(3).broadcast_to([P, T, K, D]),
                    op=mybir.AluOpType.mult)
            nc.vector.reduce_sum(staging[:, toff:toff + T, 0:heads * D],
                                 m[:].rearrange("p (t k) d -> p t k d", t=T)
                                 .transpose([0, 1, 3, 2]),
                                 axis=mybir.AxisListType.X)
            nc.vector.reduce_sum(staging[:, toff:toff + T, heads * D:],
                                 x[:].transpose([0, 1, 3, 2]),
                                 axis=mybir.AxisListType.X)
            gcol += C * 8
            mcol += C
            sgoff += C
        nc.gpsimd.dma_scatter_add(
            acc[:, :HV], staging[:], sit[:], Tsg * P, Tsg * P,
            HV, elem_step=P, single_packet=False, queue_num=3)
        ecol += Tsg * 8


def _leaky_exp(nc, pool, src_ap, heads, tagp, scale_ap=None):
    """x = exp(leaky_relu(src)) [* scale]; src consumed as f32 AP [P, heads].

    All on the (idle) scalar engine: Lrelu -> Exp -> optional scaled copy.
    """
    xs = pool.tile([P, heads], f32, tag=tagp + "xs")
    nc.scalar.activation(xs[:], src_ap, mybir.ActivationFunctionType.Exp)
    qs = pool.tile([P, heads], f32, tag=tagp + "qs")
    nc.scalar.activation(qs[:], src_ap, mybir.ActivationFunctionType.Exp,
                         scale=NEG_SLOPE)
    nc.vector.tensor_tensor(out=xs[:], in0=xs[:], in1=qs[:],
                            op=mybir.AluOpType.max)
    if scale_ap is not None:
        nc.scalar.activation(xs[:], xs[:], mybir.ActivationFunctionType.Copy,
                             scale=scale_ap)
    return xs


def build_program(plan):
    nc = bacc.Bacc("TRN2", target_bir_lowering=False, debug=False,
                   num_devices=N_CORES, num_swdge_queues=4)

    featsh = nc.dram_tensor("featsh", [OWN, IN_DIM], f32, kind="ExternalInput")
    w1ext = nc.dram_tensor("w1ext", [IN_DIM, HD + 2 * HEADS], f32,
                           kind="ExternalInput")
    w2ext = nc.dram_tensor("w2ext", [HD, HID + 2], f32, kind="ExternalInput")
    b1rep = nc.dram_tensor("b1rep", [P, HD], f32, kind="ExternalInput")
    b2rep = nc.dram_tensor("b2rep", [P, HID], f32, kind="ExternalInput")
    selfcnt_d = nc.dram_tensor("selfcnt", [P, OWN_TILES], f32,
                               kind="ExternalInput")
    gi = nc.dram_tensor("gidx", [P, max(plan.gidx_cols, 16)], i16,
                        kind="ExternalInput")
    ei = nc.dram_tensor("eidx", [P, max(plan.eidx_cols, 16)], i16,
                        kind="ExternalInput")
    si = nc.dram_tensor("sidx", [P, max(plan.eidx_cols, 16)], i16,
                        kind="ExternalInput")
    mi = nc.dram_tensor("mask", [P, max(plan.total_gcols, 1)], f32,
                        kind="ExternalInput")
    HACC = HCHUNK + P            # per-half acc rows (+ garbage region)
    acc1a = nc.dram_tensor("acc1a", [HACC, P], f32, kind="ExternalInput")
    acc1b = nc.dram_tensor("acc1b", [HACC, P], f32, kind="ExternalInput")
    acc2a = nc.dram_tensor("acc2a", [HACC, P], f32, kind="ExternalInput")
    acc2b = nc.dram_tensor("acc2b", [HACC, P], f32, kind="ExternalInput")
    out2 = nc.dram_tensor("out2", [OWN, HID], f32, kind="ExternalOutput")

    with tile.TileContext(nc) as tc:
        nc.gpsimd.load_library(library_config.mlp)
        with tc.tile_pool(name="const", bufs=1) as constp, \
             tc.tile_pool(name="gpx", bufs=6) as gpoolx, \
             tc.tile_pool(name="sbuf", bufs=3) as pool, \
             tc.tile_pool(name="psum", bufs=2, space="PSUM") as psum, \
             tc.tile_pool(name="dram", bufs=1, space="DRAM") as dram:

            own1 = dram.tile([OWN, ROW1], bf16)
            table1 = dram.tile([GLOB, ROW1], bf16)
            own2 = dram.tile([OWN, ROW2], f32)
            table2 = dram.tile([GLOB, ROW2], f32)

            ident = constp.tile([P, P], bf16)
            make_identity(nc, ident[:])
            w1t = constp.tile([IN_DIM, HD + 2 * HEADS], f32)
            nc.sync.dma_start(w1t[:], w1ext[:])
            w1b = constp.tile([IN_DIM, HD + 2 * HEADS], bf16)
            nc.vector.tensor_copy(w1b[:], w1t[:])
            w2t = constp.tile([HD, HID + 2], f32)
            nc.sync.dma_start(w2t[:], w2ext[:])
            w2b = constp.tile([HD, HID + 2], bf16)
            nc.vector.tensor_copy(w2b[:], w2t[:])
            b1t = constp.tile([P, HD], f32)
            nc.sync.dma_start(b1t[:], b1rep[:])
            b2t = constp.tile([P, HID], f32)
            nc.sync.dma_start(b2t[:], b2rep[:])
            selfc = constp.tile([P, OWN_TILES], f32)
            nc.sync.dma_start(selfc[:], selfcnt_d[:])

            HALF_TILES = OWN_TILES // 2
            SG_SPLIT = next(si for si, (pb, _t, _g)
                            in enumerate(plan.supergroups) if pb >= 4)

            def phase_a_tiles(lo, hi):
                for i in range(lo, hi):
                    r0 = i * P
                    ft = pool.tile([P, IN_DIM], f32, tag="ft")
                    nc.sync.dma_start(ft[:], featsh[r0:r0 + P, :])
                    fb = pool.tile([P, IN_DIM], bf16, tag="fb")
                    nc.vector.tensor_copy(fb[:], ft[:])
                    pt = psum.tile([P, P], bf16, tag="pt")
                    nc.tensor.transpose(out=pt[:], in_=fb[:],
                                        identity=ident[:])
                    fT = pool.tile([P, P], bf16, tag="fT")
                    nc.scalar.copy(fT[:], pt[:])
                    ph = psum.tile([P, HD + 2 * HEADS], f32, tag="ph")
                    nc.tensor.matmul(ph[:], lhsT=fT[:], rhs=w1b[:],
                                     start=True, stop=True)
                    rowt = pool.tile([P, ROW1], bf16, tag="rowt")
                    nc.vector.memset(rowt[:], 0.0)
                    nc.vector.tensor_copy(rowt[:, :HD + 2 * HEADS], ph[:])
                    nc.sync.dma_start(own1[r0:r0 + P, :], rowt[:])

            def allgather(own_t, table_t, ci):
                nc.gpsimd.collective_compute(
                    "AllGather", mybir.AluOpType.bypass,
                    replica_groups=[list(range(N_CORES))],
                    ins=[own_t[ci * HCHUNK:(ci + 1) * HCHUNK, :].opt()],
                    outs=[table_t[ci * N_CORES * HCHUNK:
                                  (ci + 1) * N_CORES * HCHUNK, :].opt()])

            # ---- Phase A interleaved with chunked AllGather-1 ----
            phase_a_tiles(0, HALF_TILES)
            allgather(own1, table1, 0)
            phase_a_tiles(HALF_TILES, OWN_TILES)
            allgather(own1, table1, 1)

            # ---- layer-1 edges ----
            qstate = [0]
            _edge_phase(nc, pool, plan, gpool=gpoolx,
                        table_full=table1, table_own=own1,
                        accs=(acc1a, acc1b),
                        gidx_d=gi, eidx_d=ei, sidx_d=si, mask_d=mi,
                        row_w=ROW1, row_dt=bf16, el_col=EL1, heads=HEADS,
                        tagp="e1", qstate=qstate)

            # ---- normalize 1 + layer-2 rows ----
            def norm1_tile(i):
                acc_t = acc1a if i < HALF_TILES else acc1b
                rr = (i % HALF_TILES) * P
                r0 = i * P
                at = pool.tile([P, HD + HEADS], f32, tag="n1at")
                nc.sync.dma_start(at[:], acc_t[rr:rr + P, :HD + HEADS])
                ownt = pool.tile([P, ROW1], bf16, tag="n1own")
                nc.sync.dma_start(ownt[:], own1[r0:r0 + P, :])
                tsum = pool.tile([P, HEADS], f32, tag="n1sum")
                nc.vector.tensor_tensor(
                    out=tsum[:], in0=ownt[:, EL1:EL1 + HEADS],
                    in1=ownt[:, EL1 + HEADS:EL1 + 2 * HEADS],
                    op=mybir.AluOpType.add)
                xs = _leaky_exp(nc, pool, tsum[:], HEADS, "n1",
                                scale_ap=selfc[:, i:i + 1])
                stot = pool.tile([P, HEADS], f32, tag="n1st")
                nc.vector.tensor_tensor(out=stot[:], in0=at[:, HD:HD + HEADS],
                                        in1=xs[:], op=mybir.AluOpType.add)
                sinv = pool.tile([P, HEADS], f32, tag="n1si")
                nc.vector.reciprocal(sinv[:], stot[:])
                o1 = pool.tile([P, HD], f32, tag="n1o")
                for h in range(HEADS):
                    sl = slice(h * HID, (h + 1) * HID)
                    nc.vector.tensor_scalar(
                        out=o1[:, sl], in0=ownt[:, sl],
                        scalar1=xs[:, h:h + 1], scalar2=None,
                        op0=mybir.AluOpType.mult)
                    nc.vector.tensor_tensor(out=o1[:, sl], in0=o1[:, sl],
                                            in1=at[:, sl],
                                            op=mybir.AluOpType.add)
                    nc.vector.tensor_scalar(
                        out=o1[:, sl], in0=o1[:, sl],
                        scalar1=sinv[:, h:h + 1], scalar2=None,
                        op0=mybir.AluOpType.mult)
                nc.vector.tensor_tensor(out=o1[:], in0=o1[:], in1=b1t[:],
                                        op=mybir.AluOpType.add)
                nc.vector.tensor_scalar(out=o1[:], in0=o1[:], scalar1=0.0,
                                        scalar2=None, op0=mybir.AluOpType.max)
                o1b = pool.tile([P, HD], bf16, tag="n1ob")
                nc.vector.tensor_copy(o1b[:], o1[:])
                pt2 = psum.tile([HD, P], bf16, tag="n1pt")
                nc.tensor.transpose(out=pt2[:], in_=o1b[:], identity=ident[:])
                oT = pool.tile([HD, P], bf16, tag="n1oT")
                nc.scalar.copy(oT[:], pt2[:])
                ph2 = psum.tile([P, HID + 2], f32, tag="n1ph")
                nc.tensor.matmul(ph2[:], lhsT=oT[:], rhs=w2b[:], start=True,
                                 stop=True)
                row2t = pool.tile([P, ROW2], f32, tag="n1r2")
                nc.vector.memset(row2t[:], 0.0)
                nc.vector.tensor_copy(row2t[:, :HID + 2], ph2[:])
                nc.sync.dma_start(own2[r0:r0 + P, :], row2t[:])

            # normalize-1 halves interleaved with chunked AllGather-2, and
            # the layer-2 edge supergroups for table-chunk-0 banks emitted
            # before the second AllGather chunk so Pool work hides it.
            for i in range(0, HALF_TILES):
                norm1_tile(i)
            allgather(own2, table2, 0)
            for i in range(HALF_TILES, OWN_TILES):
                norm1_tile(i)

            # ---- layer-2 edges (banks 0/1 first, then AG chunk 1) ----
            _edge_phase(nc, pool, plan, gpool=gpoolx,
                        table_full=table2, table_own=own2,
                        accs=(acc2a, acc2b),
                        gidx_d=gi, eidx_d=ei, sidx_d=si, mask_d=mi,
                        row_w=ROW2, row_dt=f32, el_col=EL2, heads=1,
                        tagp="e2", qstate=qstate, sg_lo=0, sg_hi=SG_SPLIT)
            allgather(own2, table2, 1)
            _edge_phase(nc, pool, plan, gpool=gpoolx,
                        table_full=table2, table_own=own2,
                        accs=(acc2a, acc2b),
                        gidx_d=gi, eidx_d=ei, sidx_d=si, mask_d=mi,
                        row_w=ROW2, row_dt=f32, el_col=EL2, heads=1,
                        tagp="e2", qstate=qstate, sg_lo=SG_SPLIT)

            # ---- normalize 2 -> out ----
            for i in range(OWN_TILES):
                acc_t2 = acc2a if i < HALF_TILES else acc2b
                rr = (i % HALF_TILES) * P
                r0 = i * P
                at = pool.tile([P, HID + 1], f32, tag="n2at")
                nc.sync.dma_start(at[:], acc_t2[rr:rr + P, :HID + 1])
                ownt = pool.tile([P, ROW2], f32, tag="n2own")
                nc.sync.dma_start(ownt[:], own2[r0:r0 + P, :])
                tsum = pool.tile([P, 1], f32, tag="n2sum")
                nc.vector.tensor_tensor(
                    out=tsum[:], in0=ownt[:, EL2:EL2 + 1],
                    in1=ownt[:, EL2 + 1:EL2 + 2], op=mybir.AluOpType.add)
                xs = _leaky_exp(nc, pool, tsum[:], 1, "n2",
                                scale_ap=selfc[:, i:i + 1])
                stot = pool.tile([P, 1], f32, tag="n2st")
                nc.vector.tensor_tensor(out=stot[:], in0=at[:, HID:HID + 1],
                                        in1=xs[:], op=mybir.AluOpType.add)
                sinv = pool.tile([P, 1], f32, tag="n2si")
                nc.vector.reciprocal(sinv[:], stot[:])
                o2 = pool.tile([P, HID], f32, tag="n2o")
                nc.vector.tensor_scalar(out=o2[:], in0=ownt[:, :HID],
                                        scalar1=xs[:, 0:1], scalar2=None,
                                        op0=mybir.AluOpType.mult)
                nc.vector.tensor_tensor(out=o2[:], in0=o2[:], in1=at[:, :HID],
                                        op=mybir.AluOpType.add)
                nc.vector.tensor_scalar(out=o2[:], in0=o2[:],
                                        scalar1=sinv[:, 0:1], scalar2=None,
                                        op0=mybir.AluOpType.mult)
                nc.vector.tensor_tensor(out=o2[:], in0=o2[:], in1=b2t[:],
                                        op=mybir.AluOpType.add)
                nc.vector.tensor_scalar(out=o2[:], in0=o2[:], scalar1=0.0,
                                        scalar2=None, op0=mybir.AluOpType.max)
                nc.sync.dma_start(out2[r0:r0 + P, :], o2[:])

    nc.compile()
    return nc


def _make_in_maps(feat, W1, al1, ar1, b1, W2, al2, ar2, b2, plan, streams,
                  selfcnts):
    extra1 = np.zeros((IN_DIM, 2 * HEADS), np.float32)
    for h in range(HEADS):
        extra1[:, h] = W1[:, h * HID:(h + 1) * HID] @ al1[h]
        extra1[:, HEADS + h] = W1[:, h * HID:(h + 1) * HID] @ ar1[h]
    w1ext = np.concatenate([W1, extra1], axis=1).astype(np.float32)
    extra2 = np.zeros((HD, 2), np.float32)
    extra2[:, 0] = W2 @ al2[0]
    extra2[:, 1] = W2 @ ar2[0]
    w2ext = np.concatenate([W2, extra2], axis=1).astype(np.float32)
    b1rep = np.tile(b1.reshape(1, -1), (P, 1)).astype(np.float32)
    b2rep = np.tile(b2.reshape(1, -1), (P, 1)).astype(np.float32)

    featpad = np.zeros((GLOB, IN_DIM), np.float32)
    featpad[:N_NODES] = feat

    def padw(a, cols, dtype):
        out = np.zeros((P, cols), dtype)
        out[:, :a.shape[1]] = a
        return out

    acc0 = np.zeros((HCHUNK + P, P), np.float32)
    in_maps = []
    for c in range(N_CORES):
        (gw, ew, sw, mw) = streams[c]
        in_maps.append({
            "featsh": featpad[c * OWN:(c + 1) * OWN].copy(),
            "w1ext": w1ext, "w2ext": w2ext, "b1rep": b1rep, "b2rep": b2rep,
            "selfcnt": selfcnts[c],
            "gidx": padw(gw, max(plan.gidx_cols, 16), np.int16),
            "eidx": padw(ew, max(plan.eidx_cols, 16), np.int16),
            "sidx": padw(sw, max(plan.eidx_cols, 16), np.int16),
            "mask": padw(mw, max(plan.total_gcols, 1), np.float32),
            "acc1a": acc0, "acc1b": acc0, "acc2a": acc0, "acc2b": acc0,
        })
    return in_maps


def kernel(feat, W1, al1, ar1, b1, W2, al2, ar2, b2, src, dst):
    feat = np.asarray(feat, np.float32)
    W1 = np.asarray(W1, np.float32)
    W2 = np.asarray(W2, np.float32)
    al1 = np.asarray(al1, np.float32)
    ar1 = np.asarray(ar1, np.float32)
    al2 = np.asarray(al2, np.float32)
    ar2 = np.asarray(ar2, np.float32)
    b1 = np.asarray(b1, np.float32)
    b2 = np.asarray(b2, np.float32)

    plan, streams, selfcnts = _prep_host(src, dst)
    nc = build_program(plan)
    in_maps = _make_in_maps(feat, W1, al1, ar1, b1, W2, al2, ar2, b2,
                            plan, streams, selfcnts)
    import os
    trace = bool(os.environ.get("GAT_TRACE"))
    res = run_bass_kernel_spmd(nc, in_maps, core_ids=list(range(N_CORES)),
                               trace=trace)
    global LAST_RESULTS
    LAST_RESULTS = res
    out = np.zeros((N_NODES, HID), np.float32)
    for c in range(N_CORES):
        lo = c * OWN
        hi = min(lo + OWN, N_NODES)
        out[lo:hi] = res.results[c]["out2"][:hi - lo]
    return out

